# revision 2
# baseline (speedup 1.0000x reference)
"""Trainium2 Bass kernel for blockwise 8x8 DCT layer — single-pass fp16.

Reference op (per 8x8 block X of each [512,512] image):
    out[8i+a, 8j+b] = sum_{k,l} D[a,l] * D[b,k] * x[8i+k, 8j+l]

The 2D DCT of one block is a single 64x64 matmul over the flattened block:
    out_flat[8a+b] = sum_{kl} M[8a+b, 8k+l] x_flat[8k+l],
    M[8a+b, 8k+l] = D[a,l] * D[b,k]

Per core (pure data parallel over batch*channel = 12 images):
  Host packs each image into a [128, 2048] fp16 tile: partition
  p = 64*h + 8*k + l (two flattened blocks stacked), free f = 32*ih + q
  with block col iw = 2*q + h.  One matmul with the stationary weight
  W = kron(I2, M.T) (contract the full 128 partitions) produces the two
  output blocks on the same partition layout — no transpose, no second
  pass.  PSUM limits a matmul to 512 free columns, so each image is 4
  matmuls + 4 PSUM->SBUF cast-copies (alternating ACT/DVE), then one
  512 KiB DMA out.  fp16 on the wire halves HBM traffic vs f32; the
  ~358 GB/s per-core HBM limit gives a ~35.2 us floor for the
  12.6 MB/core of traffic.
"""

import math
import numpy as np

import concourse.bass as bass
import concourse.tile as tile
from concourse import bacc, mybir
from concourse.bass_utils import run_bass_kernel_spmd

N_CORES = 8
B, C, H, W_IMG = 32, 3, 512, 512
IMGS_PER_CORE = (B // N_CORES) * C  # 12
F32 = mybir.dt.float32
F16 = mybir.dt.float16


def _dct_basis_np(p=8):
    u = np.arange(p)[:, None]
    x = np.arange(p)[None, :]
    cu = np.where(u == 0, 1.0 / math.sqrt(p), math.sqrt(2.0 / p))
    return (cu * np.cos((2 * x + 1) * u * np.pi / (2 * p))).astype(np.float32)


def _build_nc(n_img, repeat=1):
    nc = bacc.Bacc("TRN2", target_bir_lowering=False, debug=False)
    x_d = nc.dram_tensor("x", [n_img, 128, 2048], F16, kind="ExternalInput")
    w_d = nc.dram_tensor("w", [128, 128], F16, kind="ExternalInput")
    y_d = nc.dram_tensor("y", [n_img, 128, 2048], F16, kind="ExternalOutput")

    with tile.TileContext(nc) as tc:
        with (
            tc.tile_pool(name="wpool", bufs=1) as wpool,
            tc.tile_pool(name="xin", bufs=3) as xin_pool,
            tc.tile_pool(name="yout", bufs=3) as yout_pool,
            tc.tile_pool(name="ps", bufs=4, space="PSUM") as ps_pool,
        ):
            w_t = wpool.tile([128, 128], F16)
            nc.sync.dma_start(w_t[:], w_d[:])

            for it in range(n_img * repeat):
                img = it % n_img
                xt = xin_pool.tile([128, 2048], F16)
                nc.sync.dma_start(xt[:], x_d[img])

                yt = yout_pool.tile([128, 2048], F16)
                for s in range(4):
                    ps = ps_pool.tile([128, 512], F32)
                    nc.tensor.matmul(
                        ps[:],
                        w_t[:],
                        xt[:, 512 * s : 512 * (s + 1)],
                        start=True,
                        stop=True,
                    )
                    dst = yt[:, 512 * s : 512 * (s + 1)]
                    if s % 2 == 0:
                        nc.scalar.copy(dst, ps[:])
                    else:
                        nc.vector.tensor_copy(dst, ps[:])

                nc.sync.dma_start(y_d[img], yt[:])

    nc.compile()
    return nc


_NC_CACHE = {}
LAST_RESULTS = None
LAST_IN_MAPS = None


def _get_nc(n_img):
    if n_img not in _NC_CACHE:
        _NC_CACHE[n_img] = _build_nc(n_img)
    return _NC_CACHE[n_img]


def _host_pack(xc):
    """[n_img, 512, 512] f32 -> [n_img, 128, 2048] fp16 device layout.

    partition p = 64*h + 8*k + l, free f = 32*ih + q, iw = 2*q + h.
    """
    n = xc.shape[0]
    t = xc.reshape(n, 64, 8, 32, 2, 8)  # (img, ih, k, q, h, l)
    t = t.transpose(0, 4, 2, 5, 1, 3)  # (img, h, k, l, ih, q)
    return np.ascontiguousarray(t.reshape(n, 128, 2048).astype(np.float16))


def _host_unpack(yc):
    """[n_img, 128, 2048] fp16 -> [n_img, 512, 512] f32."""
    n = yc.shape[0]
    t = yc.reshape(n, 2, 8, 8, 64, 32)  # (img, h, a, b, ih, q)
    t = t.transpose(0, 4, 2, 5, 1, 3)  # (img, ih, a, q, h, b)
    return t.reshape(n, 512, 512).astype(np.float32)


def kernel(x, dct_basis=None, **_unused):
    x = np.asarray(x, dtype=np.float32)
    if dct_basis is None:
        D = _dct_basis_np()
    else:
        D = np.asarray(dct_basis, dtype=np.float32)
    # M[8a+b, 8k+l] = D[a,l] * D[b,k]; stationary operand is kron(I2, M.T)
    M = np.einsum("al,bk->abkl", D, D).reshape(64, 64)
    Wm = np.kron(np.eye(2, dtype=np.float32), M.T).astype(np.float16)
    Wm = np.ascontiguousarray(Wm)

    bsz = x.shape[0]
    per_core = bsz // N_CORES
    n_img = per_core * x.shape[1]

    nc = _get_nc(n_img)

    in_maps = []
    for c in range(N_CORES):
        xc = x[c * per_core : (c + 1) * per_core].reshape(n_img, H, W_IMG)
        in_maps.append({"x": _host_pack(xc), "w": Wm})

    global LAST_RESULTS, LAST_IN_MAPS
    LAST_IN_MAPS = in_maps
    res = run_bass_kernel_spmd(nc, in_maps, list(range(N_CORES)))
    LAST_RESULTS = res

    out = np.empty((bsz, x.shape[1], H, W_IMG), dtype=np.float32)
    for c in range(N_CORES):
        out[c * per_core : (c + 1) * per_core] = _host_unpack(
            res.results[c]["y"]
        ).reshape(per_core, x.shape[1], H, W_IMG)
    return out


if __name__ == "__main__":
    xs = np.random.randn(B, C, H, W_IMG).astype(np.float32)
    y = kernel(xs)
    print("kernel ran, output shape", y.shape)
